# revision 1
# baseline (speedup 1.0000x reference)
"""Cross-attention (B=4, N=2048, C=768, H=12, HD=64) on 8 TRN2 NeuronCores.

Sharding: core = (batch, head_group) with 4 batches x 2 groups of 6 heads
(data parallel over batch, tensor parallel over heads).  Each core computes
its group's Q/K/V projections, per-head-dim LayerNorm, attention, and a
partial output projection; the host sums the two group partials per batch
and adds the bias.

Device-side layout notes:
 - Activations are fed pre-transposed (c on partitions) so every matmul
   contracts over the partition dim without any on-device transposes.
 - q~ / k~ live as [384, 2048] (head-dim on partitions), so attention
   scores are computed transposed: S^T[k_tok, q_tok].  Softmax exp needs
   no row-max (LN bounds |S| < ~10), masked q rows are folded into the
   LN scale (rs *= mask) making their score columns exactly 0 -> uniform
   softmax, matching the reference's -1e9 row-fill semantics.
 - The softmax denominator comes free from a ones-column appended to v
   (PV matmul lhsT is [128, 65]; row 64 accumulates sum_j E[j, i]).
 - All matmuls run as float32r (full PE rate at moving dim >= 256,
   ~1e-4 relative error).  Producers write through f32r-bitcast APs to
   satisfy the compiler's "rounded to FP32r" rule.
"""

import numpy as np

import concourse.bass as bass
import concourse.mybir as mybir
from concourse import tile
from concourse import bass_utils
from concourse.tile_scheduler import N_PROCS
from concourse.vector_clock import ScopedClock, VectorClock

F32 = mybir.dt.float32
F32R = mybir.dt.float32r
AF = mybir.ActivationFunctionType
OP = mybir.AluOpType

B, N, C, H, HD = 4, 2048, 768, 12, 64
G = 2                 # head groups (tensor parallel)
HPG = H // G          # 6 heads per group
CL = HPG * HD         # 384 local channels
P = 128
CH = 512              # token chunk
NCH = N // CH         # 4
NT = CL // P          # 3 output tiles per group
CT = C // P           # 6 contraction tiles
TT = N // P           # 16 token tiles
KT_GRP = 2            # k-tiles per exp group ([128, 1024] S^T psum)
EPS = 1e-5
SCALE = HD ** -0.5
NCORES = 8

_nop_ctr = [0]


class _FixedTileContext(tile.TileContext):
    """Workaround for a walrus build that allows at most ONE sync-wait per
    instruction: split multi-wait instructions into single-wait NoOps on the
    same engine, and emit the kernel-tail drain's waits as a nop chain."""

    def _split_multiwait(self, insts):
        out = []
        for inst in insts:
            si = getattr(inst, "sync_info", None)
            waits = list(si.on_wait) if si is not None and si.on_wait else []
            if len(waits) > 1:
                eng = inst.engine
                for w in waits[:-1]:
                    _nop_ctr[0] += 1
                    nop = mybir.InstNoOp(
                        name=f"I-waitsplit-{_nop_ctr[0]}", ins=[], outs=[]
                    )
                    nop.engine = eng
                    nop.sync_info = mybir.SyncInfo(on_wait=[w], on_update=[])
                    self.nc.register_instruction(nop)
                    out.append(nop)
                inst.sync_info = mybir.SyncInfo(
                    on_wait=[waits[-1]], on_update=list(si.on_update)
                )
            out.append(inst)
        return out

    def _lower_ordered_insts(self, ordered):
        ordered = {bb: self._split_multiwait(ins) for bb, ins in ordered.items()}
        super()._lower_ordered_insts(ordered)

    def _drain_and_barrier(self, tick_clock, wait_clock):
        gc = tick_clock.global_clock
        vals = [gc[p] for p in range(N_PROCS)]
        for p in [q for q, v in enumerate(vals) if v > 0]:
            partial = VectorClock(
                [vals[q] if q == p else 0 for q in range(N_PROCS)]
            )
            nop = self.nc.sync.nop(nofuse=True, hint="tail_drain_wait")
            wait_clock.add_sem_waits(nop.ins, ScopedClock({None: partial}))
        self.nc.sync.drain()
        self.nc.all_engine_barrier()
        assert self.sems is not None
        popped = self.nc._tile_sem_poison_stack.pop()
        assert popped is self._sem_poison
        self.nc.clear_and_free_semaphores(list(self.sems.allocated().values()))
        self.nc.all_engine_barrier()


def _mm(nc, out, lhsT, rhs, start, stop):
    nc.tensor.matmul(
        out, lhsT, rhs, start=start, stop=stop, skip_group_check=True
    )


def _body(tc, aps):
    nc = tc.nc
    qxT, kvxT, wq, wk, wv, wp, msk, colsel, bcast, ones1, vones, outT = aps

    cpool = tc.alloc_tile_pool(name="consts", bufs=1)
    bpool = tc.alloc_tile_pool(name="big", bufs=1)

    colsel_sb = cpool.tile([P, NT, HPG], F32R, name="colsel", tag="colsel")
    nc.sync.dma_start(colsel_sb[:], colsel[:])
    bcast_sb = cpool.tile([HPG, NT, P], F32R, name="bcast", tag="bcast")
    nc.sync.dma_start(bcast_sb[:], bcast[:])
    ones4_sb = cpool.tile([65, HD], F32R, name="ones4", tag="ones4")
    nc.sync.dma_start(ones4_sb[:], ones1[:])
    msk_sb = cpool.tile([HPG, N], F32, name="msk", tag="msk")
    nc.sync.dma_start(msk_sb[:], msk[:])
    eps_sb = cpool.tile([HPG, 1], F32, name="eps", tag="eps")
    nc.vector.memset(eps_sb[:], EPS)

    q_sb = [bpool.tile([P, N], F32, name=f"q{t}", tag=f"q{t}") for t in range(NT)]
    k_sb = [bpool.tile([P, N], F32, name=f"k{t}", tag=f"k{t}") for t in range(NT)]
    v_sb = bpool.tile([P, TT, HPG, HD + 1], F32, name="v", tag="v")
    den_all = bpool.tile([65, HPG * CH], F32, name="den", tag="den")

    # ---------------- phase 1: projections + layernorm ----------------
    ps_t = tc.alloc_tile_pool(name="ps1", bufs=8, space="PSUM")
    w_pool = tc.alloc_tile_pool(name="wts", bufs=1)
    xq_pool = tc.alloc_tile_pool(name="xq", bufs=3)
    xkv_pool = tc.alloc_tile_pool(name="xkv", bufs=7)
    sq_pool = tc.alloc_tile_pool(name="sq", bufs=3)
    st_pool = tc.alloc_tile_pool(name="st", bufs=2)
    if True:
        wq_sb = w_pool.tile([P, CT, CL], F32R, name="wq", tag="wq")
        nc.sync.dma_start(wq_sb[:], wq.rearrange("(ct p) m -> p ct m", p=P))
        wk_sb = w_pool.tile([P, CT, CL], F32R, name="wk", tag="wk")
        wv_sb = w_pool.tile([P, CT, CL], F32R, name="wv", tag="wv")

        def ln_chunk(xT, w_sb, dst, masked, c):
            if True:
                cs = slice(c * CH, (c + 1) * CH)
                pp = [ps_t.tile([P, CH], F32, name="pt", tag="pt") for _ in range(NT)]
                xts = []
                pool = xq_pool if masked else xkv_pool
                xtag = "xq" if masked else "xkv"
                for ct in range(CT):
                    xt = pool.tile([P, CH], F32R, name=xtag, tag=xtag)
                    nc.sync.dma_start(xt[:], xT[ct * P:(ct + 1) * P, cs])
                    xts.append(xt)
                    for t in range(NT):
                        _mm(nc, pp[t][:], w_sb[:, ct, t * P:(t + 1) * P],
                            xt[:], ct == 0, ct == CT - 1)
                sqs = []
                for t in range(NT):
                    nc.vector.tensor_copy(dst[t][:, cs].bitcast(F32R), pp[t][:])
                    sq_t = sq_pool.tile([P, CH], F32, name="sq", tag="sq")
                    nc.scalar.activation(sq_t[:].bitcast(F32R), pp[t][:], AF.Square)
                    sqs.append(sq_t)
                mu_ps = ps_t.tile([HPG, CH], F32, name="pt", tag="pt")
                for t in range(NT):
                    _mm(nc, mu_ps[:], colsel_sb[:, t, :],
                        dst[t][:, cs].bitcast(F32R), t == 0, t == NT - 1)
                ms_ps = ps_t.tile([HPG, CH], F32, name="pt", tag="pt")
                for t in range(NT):
                    _mm(nc, ms_ps[:], colsel_sb[:, t, :],
                        sqs[t][:].bitcast(F32R), t == 0, t == NT - 1)
                st = st_pool.tile([HPG, 4 * CH], F32, name="st", tag="st")
                work = st[:, 0:CH]
                rs = st[:, CH:2 * CH]
                murs = st[:, 2 * CH:3 * CH]
                mu_sb = st[:, 3 * CH:4 * CH]
                nc.vector.tensor_copy(mu_sb.bitcast(F32R), mu_ps[:])
                # var = E[x^2] - mu^2
                nc.vector.scalar_tensor_tensor(
                    work.bitcast(F32R), mu_sb, 1.0, mu_sb, OP.mult, OP.mult)
                nc.vector.tensor_tensor(
                    work.bitcast(F32R), ms_ps[:], work, OP.subtract)
                # rs = (var + eps)^-0.5 = exp(-0.5 * ln(var + eps))
                nc.scalar.activation(murs.bitcast(F32R), work, AF.Ln,
                                     bias=eps_sb[:])
                if masked:
                    nc.scalar.activation(rs.bitcast(F32R), murs, AF.Exp,
                                         scale=-0.5)
                    # fold attn scale + query mask into rs
                    nc.vector.tensor_tensor(
                        rs.bitcast(F32R), rs, msk_sb[:, cs], OP.mult)
                else:
                    nc.scalar.activation(rs.bitcast(F32R), murs, AF.Exp,
                                         scale=-0.5)
                # murs = -mu * rs
                nc.vector.scalar_tensor_tensor(
                    murs.bitcast(F32R), mu_sb, -1.0, rs, OP.mult, OP.mult)
                for t in range(NT):
                    rrep = ps_t.tile([P, CH], F32, name="pt", tag="pt")
                    _mm(nc, rrep[:], bcast_sb[:, t, :], rs.bitcast(F32R),
                        True, True)
                    mrep = ps_t.tile([P, CH], F32, name="pt", tag="pt")
                    _mm(nc, mrep[:], bcast_sb[:, t, :], murs.bitcast(F32R),
                        True, True)
                    nc.vector.tensor_tensor(
                        dst[t][:, cs].bitcast(F32R), dst[t][:, cs], rrep[:],
                        OP.mult)
                    nc.vector.tensor_tensor(
                        dst[t][:, cs].bitcast(F32R), dst[t][:, cs], mrep[:],
                        OP.add)
                if not masked:
                    # v projection reuses this chunk's kv x-tiles
                    for tl in range(CH // P):
                        tt = c * (CH // P) + tl
                        vp = ps_t.tile([P, CL], F32, name="pt", tag="pt")
                        for ct in range(CT):
                            _mm(nc, vp[:], xts[ct][:, tl * P:(tl + 1) * P],
                                wv_sb[:, ct, :], ct == 0, ct == CT - 1)
                        nc.vector.tensor_copy(
                            v_sb[:, tt, :, 0:HD].bitcast(F32R),
                            vp[:].rearrange("p (h d) -> p h d", h=HPG))

        for c in range(NCH):
            ln_chunk(qxT, wq_sb, q_sb, True, c)
            if c == 0:
                nc.sync.dma_start(
                    wk_sb[:], wk.rearrange("(ct p) m -> p ct m", p=P))
                nc.sync.dma_start(
                    wv_sb[:], wv.rearrange("(ct p) m -> p ct m", p=P))
                nc.sync.dma_start(v_sb[:, :, :, HD].bitcast(F32R), vones[:])
            ln_chunk(kvxT, wk_sb, k_sb, False, c)

    for pool in (st_pool, sq_pool, xkv_pool, xq_pool, w_pool, ps_t):
        pool.release()

    # ---------------- phase 2: attention + output projection ----------
    ps_s = tc.alloc_tile_pool(name="ps_s", bufs=2, space="PSUM")
    ps_o = tc.alloc_tile_pool(name="ps_o", bufs=2, space="PSUM")
    ps_t = tc.alloc_tile_pool(name="ps2", bufs=2, space="PSUM")
    wp_pool = tc.alloc_tile_pool(name="wp", bufs=1)
    e_pool = tc.alloc_tile_pool(name="e", bufs=4)
    o_pool = tc.alloc_tile_pool(name="o", bufs=2)
    rcp_pool = tc.alloc_tile_pool(name="rcp", bufs=2)
    out_pool = tc.alloc_tile_pool(name="ot", bufs=3)
    if True:
        wp_sb = wp_pool.tile([P, NT, C], F32R, name="wp", tag="wp")
        nc.sync.dma_start(wp_sb[:], wp.rearrange("(t p) m -> p t m", p=P))
        for qc in range(NCH):
            qs = slice(qc * CH, (qc + 1) * CH)
            o_t = [o_pool.tile([P, CH], F32, name=f"o{t}", tag=f"o{t}") for t in range(NT)]
            for h in range(HPG):
                t, off = h // 2, (h % 2) * HD
                po = ps_o.tile([HD + 1, CH], F32, name="po", tag="po")
                for kg in range(TT // KT_GRP):
                    sp = ps_s.tile([P, KT_GRP * CH], F32, name="sp", tag="sp")
                    for j in range(KT_GRP):
                        kt = kg * KT_GRP + j
                        _mm(nc, sp[:, j * CH:(j + 1) * CH],
                            k_sb[t][off:off + HD, kt * P:(kt + 1) * P].bitcast(F32R),
                            q_sb[t][off:off + HD, qs].bitcast(F32R),
                            True, True)
                    e = e_pool.tile([P, KT_GRP * CH], F32, name="e", tag="e")
                    nc.scalar.activation(e[:].bitcast(F32R), sp[:], AF.Exp)
                    for j in range(KT_GRP):
                        kt = kg * KT_GRP + j
                        _mm(nc, po[:], v_sb[:, kt, h, :].bitcast(F32R),
                            e[:, j * CH:(j + 1) * CH].bitcast(F32R),
                            kt == 0, kt == TT - 1)
                # stash denominator (po row 64) and raw O rows; the
                # normalize happens after the qc's batched reciprocal.
                db = 32 * (qc % 3)
                nc.vector.tensor_copy(
                    den_all[db:db + 1, h * CH:(h + 1) * CH].bitcast(F32R),
                    po[HD:HD + 1, :])
                nc.vector.tensor_copy(
                    o_t[t][off:off + HD, :].bitcast(F32R), po[0:HD, :])
            # batched reciprocal for all 6 heads of this qc: repack the
            # [1, 6*CH] denominator row into [32, 96] (DVE reciprocal cost
            # scales with free size only), invert, and scatter back.
            db = 32 * (qc % 3)
            dpk = rcp_pool.tile([32, HPG * CH // 32], F32, name="dpk", tag="dpk")
            nc.sync.dma_start(dpk[:], den_all[db:db + 1, :])
            rpk = rcp_pool.tile([32, HPG * CH // 32], F32, name="rpk", tag="rpk")
            nc.vector.reciprocal(rpk[:], dpk[:])
            nc.sync.dma_start(
                den_all[db:db + 1, :].bitcast(F32R), rpk[:].bitcast(F32R))
            for h in range(HPG):
                t, off = h // 2, (h % 2) * HD
                rrep = ps_t.tile([HD, CH], F32, name="pt", tag="pt")
                _mm(nc, rrep[:], ones4_sb[db:db + 1, :],
                    den_all[db:db + 1, h * CH:(h + 1) * CH].bitcast(F32R),
                    True, True)
                nc.vector.tensor_tensor(
                    o_t[t][off:off + HD, :].bitcast(F32R),
                    o_t[t][off:off + HD, :], rrep[:], OP.mult)
            for m in range(C // P):
                pp = ps_t.tile([P, CH], F32, name="pt", tag="pt")
                for t in range(NT):
                    _mm(nc, pp[:], wp_sb[:, t, m * P:(m + 1) * P],
                        o_t[t][:].bitcast(F32R), t == 0, t == NT - 1)
                ot = out_pool.tile([P, CH], F32, name="ot", tag="ot")
                nc.vector.tensor_copy(ot[:], pp[:])
                nc.sync.dma_start(outT[m * P:(m + 1) * P, qs], ot[:])

    for pool in (out_pool, rcp_pool, o_pool, e_pool, wp_pool,
                 ps_t, ps_o, ps_s, bpool, cpool):
        pool.release()


def build_bass():
    nc = bass.Bass(trn_type="TRN2", debug=False, num_devices=NCORES)
    qxT = nc.dram_tensor("qxT", [C, N], F32R, kind="ExternalInput").ap()
    kvxT = nc.dram_tensor("kvxT", [C, N], F32R, kind="ExternalInput").ap()
    wq = nc.dram_tensor("wq", [C, CL], F32R, kind="ExternalInput").ap()
    wk = nc.dram_tensor("wk", [C, CL], F32R, kind="ExternalInput").ap()
    wv = nc.dram_tensor("wv", [C, CL], F32R, kind="ExternalInput").ap()
    wp = nc.dram_tensor("wp", [CL, C], F32R, kind="ExternalInput").ap()
    msk = nc.dram_tensor("msk", [HPG, N], F32, kind="ExternalInput").ap()
    colsel = nc.dram_tensor("colsel", [P, NT, HPG], F32R,
                            kind="ExternalInput").ap()
    bcast = nc.dram_tensor("bcast", [HPG, NT, P], F32R,
                           kind="ExternalInput").ap()
    ones1 = nc.dram_tensor("ones1", [65, HD], F32R, kind="ExternalInput").ap()
    vones = nc.dram_tensor("vones", [P, TT, HPG], F32R,
                           kind="ExternalInput").ap()
    outT = nc.dram_tensor("outT", [C, N], F32, kind="ExternalOutput").ap()
    aps = (qxT, kvxT, wq, wk, wv, wp, msk, colsel, bcast, ones1, vones, outT)
    with _FixedTileContext(nc) as tc:
        _body(tc, aps)
    return nc


def make_in_maps(q_x, kv_x, attn_mask, Wq, Wkv, Wp):
    colsel = np.zeros((P, NT, HPG), np.float32)
    bcast = np.zeros((HPG, NT, P), np.float32)
    for t in range(NT):
        for pp in range(P):
            colsel[pp, t, 2 * t + pp // HD] = 1.0 / HD
            bcast[2 * t + pp // HD, t, pp] = 1.0
    ones1 = np.zeros((65, HD), np.float32)
    ones1[[0, 32, 64], :] = 1.0

    in_maps = []
    for core in range(NCORES):
        b, g = core // G, core % G
        sl = slice(g * CL, (g + 1) * CL)
        in_maps.append({
            "qxT": np.ascontiguousarray(q_x[b].T),
            "kvxT": np.ascontiguousarray(kv_x[b].T),
            "wq": np.ascontiguousarray(Wq[sl].T),
            "wk": np.ascontiguousarray(Wkv[sl].T),
            "wv": np.ascontiguousarray(Wkv[C + g * CL:C + (g + 1) * CL].T),
            "wp": np.ascontiguousarray(Wp[:, sl].T),
            "msk": np.broadcast_to(
                attn_mask[b].astype(np.float32) * SCALE, (HPG, N)).copy(),
            "colsel": colsel,
            "bcast": bcast,
            "ones1": ones1,
            "vones": np.ones((P, TT, HPG), np.float32),
        })
    return in_maps


_NC_CACHE = []


def get_nc():
    if not _NC_CACHE:
        _NC_CACHE.append(build_bass())
    return _NC_CACHE[0]


def kernel(q_x, kv_x, attn_mask, Wq, Wkv, qn_w, qn_b, kn_w, kn_b, Wp, bp,
           _profile=None):
    q_x = np.asarray(q_x, np.float32)
    kv_x = np.asarray(kv_x, np.float32)
    attn_mask = np.asarray(attn_mask)
    Wq = np.asarray(Wq, np.float32)
    Wkv = np.asarray(Wkv, np.float32)
    Wp = np.asarray(Wp, np.float32)
    bp = np.asarray(bp, np.float32)
    if not (np.all(np.asarray(qn_w) == 1) and np.all(np.asarray(qn_b) == 0)
            and np.all(np.asarray(kn_w) == 1) and np.all(np.asarray(kn_b) == 0)):
        raise NotImplementedError("kernel specialized to identity q/k norms")

    nc = get_nc()
    in_maps = make_in_maps(q_x, kv_x, attn_mask, Wq, Wkv, Wp)
    res = bass_utils.run_bass_kernel_spmd(
        nc, in_maps, core_ids=list(range(NCORES)))
    if _profile is not None:
        _profile.append(res)
    out = np.empty((B, N, C), np.float32)
    for b in range(B):
        acc = res.results[G * b]["outT"] + res.results[G * b + 1]["outT"]
        out[b] = acc.T + bp
    return out



# revision 7
# speedup vs baseline: 1.8181x; 1.8181x over previous
"""Cross-attention (B=4, N=2048, C=768, H=12, HD=64) on 8 TRN2 NeuronCores.

Sharding: core = (batch, head_group): 4 batches x 2 groups of 6 heads.
Each core computes its group's Q/K/V projections, per-head-dim LayerNorm,
attention, and a partial output projection; the host sums the two group
partials per batch and adds the bias.

Key optimizations over the fp32 baseline:
 - All matmul operands are fp16 (1 cycle/row on the PE vs 4 for fp32 mode,
   and fast-weight-load applies).  PSUM accumulation stays fp32.
 - Query-token compaction: the reference masks along QUERY rows only, and
   every masked row produces the same output (the uniform average of V,
   since softmax(-1e9 * 1) is uniform).  The host gathers the ~50% unmasked
   tokens per batch, zero-pads to MQ=1152 columns, and scatters back; one
   guaranteed-pad column (q~ = 0 -> scores 0 -> uniform softmax) supplies
   the shared masked-row output.  Attention/exp/out-proj work drops ~2x.
 - Software pipelining: scores (PE) -> exp (ACT) -> PV (PE) run with one
   k-group of lookahead so the ACT exp stream never starves; Q-projection
   and out-projection work is sliced into small "filler" units emitted
   between attention steps to hide it under the ACT-bound window.
 - Softmax denominators come free from a ones-column appended to V; no
   row-max is needed (LN bounds |S| < ~6, exp(S) fits fp16 comfortably).
 - LN scale (HD^-0.5) is folded into the rsqrt via the activation bias:
   exp(-0.5*ln(var+eps) + ln(scale)) = scale * rsqrt(var+eps).
"""

import numpy as np

import concourse.bass as bass
import concourse.mybir as mybir
from concourse import tile
from concourse import bass_utils
from concourse.tile_scheduler import N_PROCS
from concourse.vector_clock import ScopedClock, VectorClock

F32 = mybir.dt.float32
F16 = mybir.dt.float16
AF = mybir.ActivationFunctionType
OP = mybir.AluOpType

B, N, C, H, HD = 4, 2048, 768, 12, 64
G = 2                 # head groups (tensor parallel)
HPG = H // G          # 6 heads per group
CL = HPG * HD         # 384 local channels
P = 128
CH = 512              # token chunk
NCH = N // CH         # 4 k-side chunks
NT = CL // P          # 3 output tiles per group
CT = C // P           # 6 contraction tiles
TT = N // P           # 16 k token tiles
KT_GRP = 2            # k-tiles per exp group
MQ = 1152             # padded compacted q tokens (counts are ~1024+-45)
QCHUNKS = [(0, 512), (512, 512), (1024, 128)]
EPS = 1e-5
SCALE = HD ** -0.5
LNSCALE = float(np.log(SCALE))
NCORES = 8

_nop_ctr = [0]


class _FixedTileContext(tile.TileContext):
    """Workaround for a walrus build that allows at most ONE sync-wait per
    instruction: split multi-wait instructions into single-wait NoOps on the
    same engine, and emit the kernel-tail drain's waits as a nop chain."""

    def _split_multiwait(self, insts):
        out = []
        for inst in insts:
            si = getattr(inst, "sync_info", None)
            waits = list(si.on_wait) if si is not None and si.on_wait else []
            if len(waits) > 1:
                eng = inst.engine
                for w in waits[:-1]:
                    _nop_ctr[0] += 1
                    nop = mybir.InstNoOp(
                        name=f"I-waitsplit-{_nop_ctr[0]}", ins=[], outs=[]
                    )
                    nop.engine = eng
                    nop.sync_info = mybir.SyncInfo(on_wait=[w], on_update=[])
                    self.nc.register_instruction(nop)
                    out.append(nop)
                inst.sync_info = mybir.SyncInfo(
                    on_wait=[waits[-1]], on_update=list(si.on_update)
                )
            out.append(inst)
        return out

    def _lower_ordered_insts(self, ordered):
        ordered = {bb: self._split_multiwait(ins) for bb, ins in ordered.items()}
        super()._lower_ordered_insts(ordered)

    def _drain_and_barrier(self, tick_clock, wait_clock):
        gc = tick_clock.global_clock
        vals = [gc[p] for p in range(N_PROCS)]
        for p in [q for q, v in enumerate(vals) if v > 0]:
            partial = VectorClock(
                [vals[q] if q == p else 0 for q in range(N_PROCS)]
            )
            nop = self.nc.sync.nop(nofuse=True, hint="tail_drain_wait")
            wait_clock.add_sem_waits(nop.ins, ScopedClock({None: partial}))
        self.nc.sync.drain()
        self.nc.all_engine_barrier()
        assert self.sems is not None
        popped = self.nc._tile_sem_poison_stack.pop()
        assert popped is self._sem_poison
        self.nc.clear_and_free_semaphores(list(self.sems.allocated().values()))
        self.nc.all_engine_barrier()


def _mm(nc, out, lhsT, rhs, start, stop):
    nc.tensor.matmul(
        out, lhsT, rhs, start=start, stop=stop, skip_group_check=True
    )


def _body(tc, aps):
    nc = tc.nc
    qxT, kvxT, wq, wk, wv, wp, colsel, bcast, vones, outT = aps

    cpool = tc.alloc_tile_pool(name="consts", bufs=1)
    bpool = tc.alloc_tile_pool(name="big", bufs=1)
    w_pool = tc.alloc_tile_pool(name="wts", bufs=1)

    colsel_sb = cpool.tile([P, NT, HPG], F16, name="colsel", tag="colsel")
    nc.sync.dma_start(colsel_sb[:], colsel[:])
    bcast_sb = cpool.tile([HPG, NT, P], F16, name="bcast", tag="bcast")
    nc.sync.dma_start(bcast_sb[:], bcast[:])
    eps_sb = cpool.tile([HPG, 1], F32, name="eps", tag="eps")
    nc.vector.memset(eps_sb[:], EPS)
    lnq_sb = cpool.tile([HPG, 1], F32, name="lnq", tag="lnq")
    nc.vector.memset(lnq_sb[:], LNSCALE)

    wk_sb = w_pool.tile([P, CT, CL], F16, name="wk", tag="wk")
    nc.sync.dma_start(wk_sb[:], wk.rearrange("(ct p) m -> p ct m", p=P))
    wv_sb = w_pool.tile([P, CT, CL], F16, name="wv", tag="wv")
    nc.sync.dma_start(wv_sb[:], wv.rearrange("(ct p) m -> p ct m", p=P))
    wq_sb = w_pool.tile([P, CT, CL], F16, name="wq", tag="wq")
    nc.sync.dma_start(wq_sb[:], wq.rearrange("(ct p) m -> p ct m", p=P))
    wp_sb = w_pool.tile([P, NT, C], F16, name="wp", tag="wp")
    nc.sync.dma_start(wp_sb[:], wp.rearrange("(t p) m -> p t m", p=P))

    q_sb = [bpool.tile([P, MQ], F16, name=f"q{t}", tag=f"q{t}")
            for t in range(NT)]
    k_sb = [bpool.tile([P, N], F16, name=f"k{t}", tag=f"k{t}")
            for t in range(NT)]
    v_sb = bpool.tile([P, TT, HPG, HD + 1], F16, name="v", tag="v")
    nc.sync.dma_start(v_sb[:, :, :, HD], vones[:])

    # ---------------- phase A pools (k/v projections + LN) -------------
    ppA = tc.alloc_tile_pool(name="ppA", bufs=4, space="PSUM")
    stA = tc.alloc_tile_pool(name="stA", bufs=2, space="PSUM")
    rrA = tc.alloc_tile_pool(name="rrA", bufs=2, space="PSUM")
    xkv_pool = tc.alloc_tile_pool(name="xkv", bufs=12)
    sq_pool = tc.alloc_tile_pool(name="sq", bufs=6)
    st32 = tc.alloc_tile_pool(name="st32", bufs=6)
    st16 = tc.alloc_tile_pool(name="st16", bufs=4)

    def proj_ln_chunk(xT, w_sb, dst, is_q, coff, W, xpool, pp_pool, st_pool,
                      rr_pool, with_v, tags=("pp", "stp", "rr")):
        """Project one token chunk [coff, coff+W), layernorm per head-dim,
        optionally also compute the V projection from the same x tiles.
        Emission order interleaves V-proj matmuls as PE filler while the
        LN-apply DVE ops drain, keeping the psum rotation unblocked."""
        tag_pp, tag_st, tag_rr = tags
        xts = []
        for ct in range(CT):
            xt = xpool.tile([P, CH], F16, name="xt", tag="xt")
            nc.sync.dma_start(xt[:, 0:W], xT[ct * P:(ct + 1) * P,
                                             coff:coff + W])
            xts.append(xt)
        sqs = []
        for t in range(NT):
            pp = pp_pool.tile([P, CH], F32, name="pp", tag=tag_pp)
            for ct in range(CT):
                _mm(nc, pp[:, 0:W], w_sb[:, ct, t * P:(t + 1) * P],
                    xts[ct][:, 0:W], ct == 0, ct == CT - 1)
            nc.vector.tensor_copy(dst[t][:, coff:coff + W], pp[:, 0:W])
            sq = sq_pool.tile([P, CH], F16, name="sq", tag="sq")
            nc.vector.tensor_tensor(
                sq[:, 0:W], dst[t][:, coff:coff + W],
                dst[t][:, coff:coff + W], OP.mult)
            sqs.append(sq)
        mu_ps = st_pool.tile([HPG, CH], F32, name="mu_ps", tag=tag_st)
        for t in range(NT):
            _mm(nc, mu_ps[:, 0:W], colsel_sb[:, t, :],
                dst[t][:, coff:coff + W], t == 0, t == NT - 1)
        ms_ps = st_pool.tile([HPG, CH], F32, name="ms_ps", tag=tag_st)
        for t in range(NT):
            _mm(nc, ms_ps[:, 0:W], colsel_sb[:, t, :], sqs[t][:, 0:W],
                t == 0, t == NT - 1)
        mu = st32.tile([HPG, CH], F32, name="mu", tag="mu")
        nc.vector.tensor_copy(mu[:, 0:W], mu_ps[:, 0:W])
        var = st32.tile([HPG, CH], F32, name="var", tag="var")
        nc.vector.scalar_tensor_tensor(
            var[:, 0:W], mu[:, 0:W], 1.0, mu[:, 0:W], OP.mult, OP.mult)
        nc.vector.tensor_tensor(
            var[:, 0:W], ms_ps[:, 0:W], var[:, 0:W], OP.subtract)
        lnv = st32.tile([HPG, CH], F32, name="lnv", tag="lnv")
        nc.scalar.activation(lnv[:, 0:W], var[:, 0:W], AF.Ln,
                             bias=eps_sb[:])
        rs = st16.tile([HPG, CH], F16, name="rs", tag="rs")
        if is_q:
            nc.scalar.activation(rs[:, 0:W], lnv[:, 0:W], AF.Exp, scale=-0.5,
                                 bias=lnq_sb[:])
        else:
            nc.scalar.activation(rs[:, 0:W], lnv[:, 0:W], AF.Exp, scale=-0.5)
        murs = st16.tile([HPG, CH], F16, name="murs", tag="murs")
        nc.vector.scalar_tensor_tensor(
            murs[:, 0:W], mu[:, 0:W], -1.0, rs[:, 0:W], OP.mult, OP.mult)
        for t in range(NT):
            rrep = rr_pool.tile([P, CH], F32, name="rrep", tag=tag_rr)
            _mm(nc, rrep[:, 0:W], bcast_sb[:, t, :], rs[:, 0:W], True, True)
            mrep = rr_pool.tile([P, CH], F32, name="mrep", tag=tag_rr)
            _mm(nc, mrep[:, 0:W], bcast_sb[:, t, :], murs[:, 0:W],
                True, True)
            nc.vector.tensor_tensor(
                dst[t][:, coff:coff + W], dst[t][:, coff:coff + W],
                rrep[:, 0:W], OP.mult)
            nc.vector.tensor_tensor(
                dst[t][:, coff:coff + W], dst[t][:, coff:coff + W],
                mrep[:, 0:W], OP.add)
            # V-projection matmuls act as PE filler while LN-apply drains
            if with_v and t < 2:
                for tl in (t, t + 2):
                    tt = coff // P + tl
                    vp = pp_pool.tile([P, CH], F32, name="vp", tag=tag_pp)
                    for ct in range(CT):
                        _mm(nc, vp[:, 0:CL],
                            xts[ct][:, tl * P:(tl + 1) * P],
                            wv_sb[:, ct, :], ct == 0, ct == CT - 1)
                    nc.vector.tensor_copy(
                        v_sb[:, tt, :, 0:HD],
                        vp[:, 0:CL].rearrange("p (h d) -> p h d", h=HPG))

    # ---------------- phase A: K projection + LN + V -------------------
    for c in range(NCH):
        proj_ln_chunk(kvxT, wk_sb, k_sb, False, c * CH, CH,
                      xkv_pool, ppA, stA, rrA, True)

    for pool in (rrA, stA, ppA):
        pool.release()

    # ---------------- phase B pools (q proj + attention) ---------------
    ps_sp = tc.alloc_tile_pool(name="ps_sp", bufs=2, space="PSUM")
    ps_po = tc.alloc_tile_pool(name="ps_po", bufs=2, space="PSUM")
    ps_mi = tc.alloc_tile_pool(name="ps_mi", bufs=2, space="PSUM")
    xq_pool = tc.alloc_tile_pool(name="xq", bufs=12)
    e_pool = tc.alloc_tile_pool(name="e", bufs=4)
    o_pool = tc.alloc_tile_pool(name="o", bufs=6)
    den_pool = tc.alloc_tile_pool(name="den", bufs=2)
    out_pool = tc.alloc_tile_pool(name="ot", bufs=3)

    def qln(qc):
        coff, W = QCHUNKS[qc]
        proj_ln_chunk(qxT, wq_sb, q_sb, True, coff, W,
                      xq_pool, ps_mi, ps_mi, ps_mi, False,
                      tags=("misc", "misc", "misc"))

    # attention state per qc chunk
    o_t = {}     # qc -> [3 tiles [128, W] f16]
    den6 = {}    # qc -> [6, W] f32 denominators
    po_cur = [None]
    sp_state = {}   # step index -> (sp psum, e tile)

    steps = [(qc, h, kg) for qc in range(len(QCHUNKS))
             for h in range(HPG) for kg in range(TT // KT_GRP)]

    def emit_sp(i):
        qc, h, kg = steps[i]
        coff, W = QCHUNKS[qc]
        t, off = h // 2, (h % 2) * HD
        sp = ps_sp.tile([P, KT_GRP * CH], F32, name="sp", tag="sp")
        for j in range(KT_GRP):
            kt = kg * KT_GRP + j
            _mm(nc, sp[:, j * W:(j + 1) * W],
                k_sb[t][off:off + HD, kt * P:(kt + 1) * P],
                q_sb[t][off:off + HD, coff:coff + W], True, True)
        e = e_pool.tile([P, KT_GRP * CH], F16, name="e", tag="e")
        sp_state[i] = (sp, e)

    def emit_exp(i):
        sp, e = sp_state[i]
        qc, h, kg = steps[i]
        W = QCHUNKS[qc][1]
        nc.scalar.activation(e[:, 0:KT_GRP * W], sp[:, 0:KT_GRP * W], AF.Exp)

    def emit_pv(i):
        qc, h, kg = steps[i]
        W = QCHUNKS[qc][1]
        sp, e = sp_state.pop(i)
        if kg == 0:
            po_cur[0] = ps_po.tile([HD + 1, CH], F32, name="po", tag="po")
        po = po_cur[0]
        for j in range(KT_GRP):
            kt = kg * KT_GRP + j
            _mm(nc, po[:, 0:W], v_sb[:, kt, h, :], e[:, j * W:(j + 1) * W],
                kt == 0, kt == TT - 1)
        if kg == TT // KT_GRP - 1:
            # stash denominator (row HD) and raw O rows
            if h == 0:
                o_t[qc] = [o_pool.tile([P, CH], F16, name="ot", tag="ot")
                           for _ in range(NT)]
                den6[qc] = den_pool.tile([1, HPG * CH], F32, name="d6",
                                         tag="d6")
            t, off = h // 2, (h % 2) * HD
            nc.vector.tensor_copy(
                den6[qc][0:1, h * W:(h + 1) * W], po[HD:HD + 1, 0:W])
            nc.vector.tensor_copy(
                o_t[qc][t][off:off + HD, 0:W], po[0:HD, 0:W])

    def norm_outproj_units(qc):
        """Normalize by softmax denominators and project out; returns a
        list of small emission units to spread across attention steps."""
        coff, W = QCHUNKS[qc]
        units = []
        d6p = den_pool.tile([HPG, CH], F32, name="d6p", tag="d6p")
        d6r = den_pool.tile([HPG, CH], F16, name="d6r", tag="d6r")

        def recip():
            # scatter the staged [1, 6W] denominator row across partitions
            nc.sync.dma_start(d6p[0:HPG, 0:W], den6[qc][0:1, 0:HPG * W])
            with nc.allow_low_precision(reason="softmax denom recip in f16"):
                nc.vector.reciprocal(d6r[0:HPG, 0:W], d6p[0:HPG, 0:W])
        units.append(recip)
        for t in range(NT):
            def norm(t=t):
                rrep = ps_mi.tile([P, CH], F32, name="nrr", tag="misc")
                _mm(nc, rrep[:, 0:W], bcast_sb[:, t, :], d6r[:, 0:W],
                    True, True)
                nc.vector.tensor_tensor(
                    o_t[qc][t][:, 0:W], o_t[qc][t][:, 0:W], rrep[:, 0:W],
                    OP.mult)
            units.append(norm)
        for m in range(CT):
            def oproj(m=m):
                pp = ps_mi.tile([P, CH], F32, name="opp", tag="misc")
                for t in range(NT):
                    _mm(nc, pp[:, 0:W], wp_sb[:, t, m * P:(m + 1) * P],
                        o_t[qc][t][:, 0:W], t == 0, t == NT - 1)
                ot = out_pool.tile([P, CH], F32, name="oc", tag="oc")
                nc.vector.tensor_copy(ot[:, 0:W], pp[:, 0:W])
                nc.sync.dma_start(outT[m * P:(m + 1) * P, coff:coff + W],
                                  ot[:, 0:W])
            units.append(oproj)
        return units

    # ---------------- phase B emission with software pipelining --------
    qln(0)
    qln(1)
    fillers = []
    emit_sp(0)
    for i, (qc, h, kg) in enumerate(steps):
        if i + 1 < len(steps):
            emit_sp(i + 1)
        emit_exp(i)
        # qc boundary: queue up filler units for the window we just entered
        if kg == 0 and h == 0:
            if qc == 0:
                fillers.extend([lambda: qln(2)])
            else:
                fillers.extend(norm_outproj_units(qc - 1))
        emit_pv(i)
        if i % 4 == 3 and fillers:
            fillers.pop(0)()
    while fillers:
        fillers.pop(0)()
    for unit in norm_outproj_units(len(QCHUNKS) - 1):
        unit()

    for pool in (out_pool, den_pool, o_pool, e_pool, xq_pool,
                 ps_mi, ps_po, ps_sp, st16, st32, sq_pool, xkv_pool,
                 w_pool, bpool, cpool):
        pool.release()


def build_bass():
    nc = bass.Bass(trn_type="TRN2", debug=False, num_devices=NCORES)
    qxT = nc.dram_tensor("qxT", [C, MQ], F16, kind="ExternalInput").ap()
    kvxT = nc.dram_tensor("kvxT", [C, N], F16, kind="ExternalInput").ap()
    wq = nc.dram_tensor("wq", [C, CL], F16, kind="ExternalInput").ap()
    wk = nc.dram_tensor("wk", [C, CL], F16, kind="ExternalInput").ap()
    wv = nc.dram_tensor("wv", [C, CL], F16, kind="ExternalInput").ap()
    wp = nc.dram_tensor("wp", [CL, C], F16, kind="ExternalInput").ap()
    colsel = nc.dram_tensor("colsel", [P, NT, HPG], F16,
                            kind="ExternalInput").ap()
    bcast = nc.dram_tensor("bcast", [HPG, NT, P], F16,
                           kind="ExternalInput").ap()
    vones = nc.dram_tensor("vones", [P, TT, HPG], F16,
                           kind="ExternalInput").ap()
    outT = nc.dram_tensor("outT", [C, MQ], F32, kind="ExternalOutput").ap()
    aps = (qxT, kvxT, wq, wk, wv, wp, colsel, bcast, vones, outT)
    with _FixedTileContext(nc) as tc:
        _body(tc, aps)
    return nc


def make_in_maps(q_x, kv_x, attn_mask, Wq, Wkv, Wp):
    colsel = np.zeros((P, NT, HPG), np.float16)
    bcast = np.zeros((HPG, NT, P), np.float16)
    for t in range(NT):
        for pp in range(P):
            colsel[pp, t, 2 * t + pp // HD] = 1.0 / HD
            bcast[2 * t + pp // HD, t, pp] = 1.0

    mask = np.asarray(attn_mask, bool)
    in_maps = []
    for core in range(NCORES):
        b, g = core // G, core % G
        sl = slice(g * CL, (g + 1) * CL)
        idx = np.flatnonzero(mask[b])
        cnt = len(idx)
        assert cnt < MQ, f"mask count {cnt} exceeds padded width {MQ}"
        qxT_c = np.zeros((C, MQ), np.float16)
        qxT_c[:, :cnt] = q_x[b][idx].T
        in_maps.append({
            "qxT": qxT_c,
            "kvxT": np.ascontiguousarray(kv_x[b].T.astype(np.float16)),
            "wq": np.ascontiguousarray(Wq[sl].T.astype(np.float16)),
            "wk": np.ascontiguousarray(Wkv[sl].T.astype(np.float16)),
            "wv": np.ascontiguousarray(
                Wkv[C + g * CL:C + (g + 1) * CL].T.astype(np.float16)),
            "wp": np.ascontiguousarray(Wp[:, sl].T.astype(np.float16)),
            "colsel": colsel,
            "bcast": bcast,
            "vones": np.ones((P, TT, HPG), np.float16),
        })
    return in_maps


_NC_CACHE = []


def get_nc():
    if not _NC_CACHE:
        _NC_CACHE.append(build_bass())
    return _NC_CACHE[0]


def kernel(q_x, kv_x, attn_mask, Wq, Wkv, qn_w, qn_b, kn_w, kn_b, Wp, bp,
           _profile=None):
    q_x = np.asarray(q_x, np.float32)
    kv_x = np.asarray(kv_x, np.float32)
    attn_mask = np.asarray(attn_mask, bool)
    Wq = np.asarray(Wq, np.float32)
    Wkv = np.asarray(Wkv, np.float32)
    Wp = np.asarray(Wp, np.float32)
    bp = np.asarray(bp, np.float32)
    if not (np.all(np.asarray(qn_w) == 1) and np.all(np.asarray(qn_b) == 0)
            and np.all(np.asarray(kn_w) == 1) and np.all(np.asarray(kn_b) == 0)):
        raise NotImplementedError("kernel specialized to identity q/k norms")

    nc = get_nc()
    in_maps = make_in_maps(q_x, kv_x, attn_mask, Wq, Wkv, Wp)
    res = bass_utils.run_bass_kernel_spmd(
        nc, in_maps, core_ids=list(range(NCORES)))
    if _profile is not None:
        _profile.append(res)
    out = np.empty((B, N, C), np.float32)
    for b in range(B):
        acc = res.results[G * b]["outT"] + res.results[G * b + 1]["outT"]
        idx = np.flatnonzero(attn_mask[b])
        cnt = len(idx)
        out[b, idx] = acc[:, :cnt].T + bp
        out[b, ~attn_mask[b]] = acc[:, cnt] + bp
    return out


# revision 10
# speedup vs baseline: 2.2367x; 1.2302x over previous
"""Cross-attention (B=4, N=2048, C=768, H=12, HD=64) on 8 TRN2 NeuronCores.

Sharding: core = (batch, head_group): 4 batches x 2 groups of 6 heads.
Each core computes its group's Q/K/V projections, per-head-dim LayerNorm,
attention, and a partial output projection; the host sums the two group
partials per batch and adds the bias.

Key optimizations over the fp32 baseline:
 - All matmul operands are fp16 (1 cycle/row on the PE vs 4 for fp32 mode,
   and fast-weight-load applies).  PSUM accumulation stays fp32.
 - Query-token compaction: the reference masks along QUERY rows only, and
   every masked row produces the same output (the uniform average of V,
   since softmax(-1e9 * 1) is uniform).  The host gathers the ~50% unmasked
   tokens per batch, zero-pads to MQ=1152 columns, and scatters back; one
   guaranteed-pad column (q~ = 0 -> scores 0 -> uniform softmax) supplies
   the shared masked-row output.  Attention/exp/out-proj work drops ~2x.
 - Software pipelining: scores (PE) -> exp (ACT) -> PV (PE) run with one
   k-group of lookahead so the ACT exp stream never starves; Q-projection
   and out-projection work is sliced into small "filler" units emitted
   between attention steps to hide it under the ACT-bound window.
 - Softmax denominators come free from a ones-column appended to V; no
   row-max is needed (LN bounds |S| < ~6, exp(S) fits fp16 comfortably).
 - LN scale (HD^-0.5) is folded into the rsqrt via the activation bias:
   exp(-0.5*ln(var+eps) + ln(scale)) = scale * rsqrt(var+eps).
"""

import numpy as np

import concourse.bass as bass
import concourse.mybir as mybir
from concourse import tile
from concourse import bass_utils
from concourse.tile_scheduler import N_PROCS
from concourse.vector_clock import ScopedClock, VectorClock

F32 = mybir.dt.float32
F16 = mybir.dt.float16
AF = mybir.ActivationFunctionType
OP = mybir.AluOpType

B, N, C, H, HD = 4, 2048, 768, 12, 64
G = 2                 # head groups (tensor parallel)
HPG = H // G          # 6 heads per group
CL = HPG * HD         # 384 local channels
P = 128
CH = 512              # token chunk
NCH = N // CH         # 4 k-side chunks
NT = CL // P          # 3 output tiles per group
CT = C // P           # 6 contraction tiles
TT = N // P           # 16 k token tiles
KT_GRP = 2            # k-tiles per exp group
MQ = 1152             # padded compacted q tokens (counts are ~1024+-45)
QCHUNKS = [(0, 512), (512, 512), (1024, 128)]
EPS = 1e-5
SCALE = HD ** -0.5
LNSCALE = float(np.log(SCALE))
NCORES = 8

_nop_ctr = [0]


class _FixedTileContext(tile.TileContext):
    """Workaround for a walrus build that allows at most ONE sync-wait per
    instruction: split multi-wait instructions into single-wait NoOps on the
    same engine, and emit the kernel-tail drain's waits as a nop chain."""

    def _split_multiwait(self, insts):
        out = []
        for inst in insts:
            si = getattr(inst, "sync_info", None)
            waits = list(si.on_wait) if si is not None and si.on_wait else []
            if len(waits) > 1:
                eng = inst.engine
                for w in waits[:-1]:
                    _nop_ctr[0] += 1
                    nop = mybir.InstNoOp(
                        name=f"I-waitsplit-{_nop_ctr[0]}", ins=[], outs=[]
                    )
                    nop.engine = eng
                    nop.sync_info = mybir.SyncInfo(on_wait=[w], on_update=[])
                    self.nc.register_instruction(nop)
                    out.append(nop)
                inst.sync_info = mybir.SyncInfo(
                    on_wait=[waits[-1]], on_update=list(si.on_update)
                )
            out.append(inst)
        return out

    def _lower_ordered_insts(self, ordered):
        ordered = {bb: self._split_multiwait(ins) for bb, ins in ordered.items()}
        super()._lower_ordered_insts(ordered)

    def _drain_and_barrier(self, tick_clock, wait_clock):
        gc = tick_clock.global_clock
        vals = [gc[p] for p in range(N_PROCS)]
        for p in [q for q, v in enumerate(vals) if v > 0]:
            partial = VectorClock(
                [vals[q] if q == p else 0 for q in range(N_PROCS)]
            )
            nop = self.nc.sync.nop(nofuse=True, hint="tail_drain_wait")
            wait_clock.add_sem_waits(nop.ins, ScopedClock({None: partial}))
        self.nc.sync.drain()
        self.nc.all_engine_barrier()
        assert self.sems is not None
        popped = self.nc._tile_sem_poison_stack.pop()
        assert popped is self._sem_poison
        self.nc.clear_and_free_semaphores(list(self.sems.allocated().values()))
        self.nc.all_engine_barrier()


def _mm(nc, out, lhsT, rhs, start, stop):
    nc.tensor.matmul(
        out, lhsT, rhs, start=start, stop=stop, skip_group_check=True
    )


def _body(tc, aps):
    nc = tc.nc
    qxT, kvxT, wq, wk, wv, wp, colsel, bcast, outT = aps

    cpool = tc.alloc_tile_pool(name="consts", bufs=1)
    bpool = tc.alloc_tile_pool(name="big", bufs=1)
    w_pool = tc.alloc_tile_pool(name="wts", bufs=1)

    colsel_sb = cpool.tile([P, NT, HPG], F16, name="colsel", tag="colsel")
    nc.sync.dma_start(colsel_sb[:], colsel[:])
    bcast_sb = cpool.tile([HPG, NT, P], F16, name="bcast", tag="bcast")
    nc.sync.dma_start(bcast_sb[:], bcast[:])
    eps_sb = cpool.tile([HPG, 1], F32, name="eps", tag="eps")
    nc.vector.memset(eps_sb[:], EPS)
    lnq_sb = cpool.tile([HPG, 1], F32, name="lnq", tag="lnq")
    nc.vector.memset(lnq_sb[:], LNSCALE)

    wk_sb = w_pool.tile([P, CT, CL], F16, name="wk", tag="wk")
    nc.sync.dma_start(wk_sb[:], wk.rearrange("(ct p) m -> p ct m", p=P))
    wv_sb = w_pool.tile([P, CT, CL], F16, name="wv", tag="wv")
    nc.sync.dma_start(wv_sb[:], wv.rearrange("(ct p) m -> p ct m", p=P))
    wq_sb = w_pool.tile([P, CT, CL], F16, name="wq", tag="wq")
    wp_sb = w_pool.tile([P, NT, C], F16, name="wp", tag="wp")

    q_sb = [bpool.tile([P, MQ], F16, name=f"q{t}", tag=f"q{t}")
            for t in range(NT)]
    k_sb = [bpool.tile([P, N], F16, name=f"k{t}", tag=f"k{t}")
            for t in range(NT)]
    v_sb = bpool.tile([P, TT, HPG, HD + 1], F16, name="v", tag="v")
    nc.vector.memset(v_sb[:, :, :, HD], 1.0)

    # ---------------- phase A pools (k/v projections + LN) -------------
    ppA = tc.alloc_tile_pool(name="ppA", bufs=4, space="PSUM")
    stA = tc.alloc_tile_pool(name="stA", bufs=2, space="PSUM")
    rrA = tc.alloc_tile_pool(name="rrA", bufs=2, space="PSUM")
    xkv_pool = tc.alloc_tile_pool(name="xkv", bufs=12)
    sq_pool = tc.alloc_tile_pool(name="sq", bufs=6)
    st32 = tc.alloc_tile_pool(name="st32", bufs=6)
    st16 = tc.alloc_tile_pool(name="st16", bufs=4)

    def proj_ln_chunk(xT, w_sb, dst, is_q, coff, W, xpool, pp_pool, st_pool,
                      rr_pool, with_v, tags=("pp", "stp", "rr")):
        """Project one token chunk [coff, coff+W), layernorm per head-dim,
        optionally also compute the V projection from the same x tiles.
        Emission order interleaves V-proj matmuls as PE filler while the
        LN-apply DVE ops drain, keeping the psum rotation unblocked."""
        tag_pp, tag_st, tag_rr = tags
        xts = []
        for ct in range(CT):
            xt = xpool.tile([P, CH], F16, name="xt", tag="xt")
            nc.sync.dma_start(xt[:, 0:W], xT[ct * P:(ct + 1) * P,
                                             coff:coff + W])
            xts.append(xt)
        sqs = []
        for t in range(NT):
            pp = pp_pool.tile([P, CH], F32, name="pp", tag=tag_pp)
            for ct in range(CT):
                _mm(nc, pp[:, 0:W], w_sb[:, ct, t * P:(t + 1) * P],
                    xts[ct][:, 0:W], ct == 0, ct == CT - 1)
            nc.vector.tensor_copy(dst[t][:, coff:coff + W], pp[:, 0:W])
            sq = sq_pool.tile([P, CH], F16, name="sq", tag="sq")
            nc.scalar.activation(sq[:, 0:W], pp[:, 0:W], AF.Square)
            sqs.append(sq)
        mu_ps = st_pool.tile([HPG, CH], F32, name="mu_ps", tag=tag_st)
        for t in range(NT):
            _mm(nc, mu_ps[:, 0:W], colsel_sb[:, t, :],
                dst[t][:, coff:coff + W], t == 0, t == NT - 1)
        ms_ps = st_pool.tile([HPG, CH], F32, name="ms_ps", tag=tag_st)
        for t in range(NT):
            _mm(nc, ms_ps[:, 0:W], colsel_sb[:, t, :], sqs[t][:, 0:W],
                t == 0, t == NT - 1)
        mu = st32.tile([HPG, CH], F32, name="mu", tag="mu")
        nc.vector.tensor_copy(mu[:, 0:W], mu_ps[:, 0:W])
        var = st32.tile([HPG, CH], F32, name="var", tag="var")
        nc.vector.scalar_tensor_tensor(
            var[:, 0:W], mu[:, 0:W], 1.0, mu[:, 0:W], OP.mult, OP.mult)
        nc.vector.tensor_tensor(
            var[:, 0:W], ms_ps[:, 0:W], var[:, 0:W], OP.subtract)
        lnv = st32.tile([HPG, CH], F32, name="lnv", tag="lnv")
        nc.scalar.activation(lnv[:, 0:W], var[:, 0:W], AF.Ln,
                             bias=eps_sb[:])
        rs = st16.tile([HPG, CH], F16, name="rs", tag="rs")
        if is_q:
            nc.scalar.activation(rs[:, 0:W], lnv[:, 0:W], AF.Exp, scale=-0.5,
                                 bias=lnq_sb[:])
        else:
            nc.scalar.activation(rs[:, 0:W], lnv[:, 0:W], AF.Exp, scale=-0.5)
        murs = st16.tile([HPG, CH], F16, name="murs", tag="murs")
        nc.vector.scalar_tensor_tensor(
            murs[:, 0:W], mu[:, 0:W], -1.0, rs[:, 0:W], OP.mult, OP.mult)
        for t in range(NT):
            rrep = rr_pool.tile([P, CH], F32, name="rrep", tag=tag_rr)
            _mm(nc, rrep[:, 0:W], bcast_sb[:, t, :], rs[:, 0:W], True, True)
            mrep = rr_pool.tile([P, CH], F32, name="mrep", tag=tag_rr)
            _mm(nc, mrep[:, 0:W], bcast_sb[:, t, :], murs[:, 0:W],
                True, True)
            nc.vector.tensor_tensor(
                dst[t][:, coff:coff + W], dst[t][:, coff:coff + W],
                rrep[:, 0:W], OP.mult)
            nc.vector.tensor_tensor(
                dst[t][:, coff:coff + W], dst[t][:, coff:coff + W],
                mrep[:, 0:W], OP.add)
            # V-projection matmuls act as PE filler while LN-apply drains
            if with_v and t < 2:
                for tl in (t, t + 2):
                    tt = coff // P + tl
                    vp = pp_pool.tile([P, CH], F32, name="vp", tag=tag_pp)
                    for ct in range(CT):
                        _mm(nc, vp[:, 0:CL],
                            xts[ct][:, tl * P:(tl + 1) * P],
                            wv_sb[:, ct, :], ct == 0, ct == CT - 1)
                    nc.scalar.activation(
                        v_sb[:, tt, :, 0:HD],
                        vp[:, 0:CL].rearrange("p (h d) -> p h d", h=HPG),
                        AF.Copy)

    # ---------------- phase A: K projection + LN + V -------------------
    for c in range(NCH):
        proj_ln_chunk(kvxT, wk_sb, k_sb, False, c * CH, CH,
                      xkv_pool, ppA, stA, rrA, True)
        if c == 0:
            nc.sync.dma_start(
                wq_sb[:], wq.rearrange("(ct p) m -> p ct m", p=P))
            nc.sync.dma_start(
                wp_sb[:], wp.rearrange("(t p) m -> p t m", p=P))

    for pool in (rrA, stA, ppA):
        pool.release()

    # ---------------- phase B pools (q proj + attention) ---------------
    ps_sp = tc.alloc_tile_pool(name="ps_sp", bufs=2, space="PSUM")
    ps_po = tc.alloc_tile_pool(name="ps_po", bufs=2, space="PSUM")
    ps_mi = tc.alloc_tile_pool(name="ps_mi", bufs=2, space="PSUM")
    xq_pool = tc.alloc_tile_pool(name="xq", bufs=12)
    e_pool = tc.alloc_tile_pool(name="e", bufs=4)
    o_pool = tc.alloc_tile_pool(name="o", bufs=6)
    den_pool = tc.alloc_tile_pool(name="den", bufs=2)
    out_pool = tc.alloc_tile_pool(name="ot", bufs=3)

    def qln(qc):
        coff, W = QCHUNKS[qc]
        proj_ln_chunk(qxT, wq_sb, q_sb, True, coff, W,
                      xq_pool, ps_mi, ps_mi, ps_mi, False,
                      tags=("misc", "misc", "misc"))

    def qln_units(qc):
        """proj_ln_chunk split into ~1us emission units so it can be
        spread across attention steps without starving the ACT engine."""
        coff, W = QCHUNKS[qc]
        st = {"xts": [], "sqs": [None] * NT}
        units = []

        def dmas():
            for ct in range(CT):
                xt = xq_pool.tile([P, CH], F16, name="xt", tag="xt")
                nc.sync.dma_start(
                    xt[:, 0:W], qxT[ct * P:(ct + 1) * P, coff:coff + W])
                st["xts"].append(xt)
        units.append(dmas)
        for t in range(NT):
            def proj(t=t):
                pp = ps_mi.tile([P, CH], F32, name="pp", tag="misc")
                for ct in range(CT):
                    _mm(nc, pp[:, 0:W], wq_sb[:, ct, t * P:(t + 1) * P],
                        st["xts"][ct][:, 0:W], ct == 0, ct == CT - 1)
                nc.vector.tensor_copy(q_sb[t][:, coff:coff + W], pp[:, 0:W])
                sq = sq_pool.tile([P, CH], F16, name="sq", tag="sq")
                nc.scalar.activation(sq[:, 0:W], pp[:, 0:W], AF.Square)
                st["sqs"][t] = sq
            units.append(proj)

        def stats():
            mu_ps = ps_mi.tile([HPG, CH], F32, name="mu_ps", tag="misc")
            for t in range(NT):
                _mm(nc, mu_ps[:, 0:W], colsel_sb[:, t, :],
                    q_sb[t][:, coff:coff + W], t == 0, t == NT - 1)
            ms_ps = ps_mi.tile([HPG, CH], F32, name="ms_ps", tag="misc")
            for t in range(NT):
                _mm(nc, ms_ps[:, 0:W], colsel_sb[:, t, :],
                    st["sqs"][t][:, 0:W], t == 0, t == NT - 1)
            mu = st32.tile([HPG, CH], F32, name="mu", tag="mu")
            nc.vector.tensor_copy(mu[:, 0:W], mu_ps[:, 0:W])
            var = st32.tile([HPG, CH], F32, name="var", tag="var")
            nc.vector.scalar_tensor_tensor(
                var[:, 0:W], mu[:, 0:W], 1.0, mu[:, 0:W], OP.mult, OP.mult)
            nc.vector.tensor_tensor(
                var[:, 0:W], ms_ps[:, 0:W], var[:, 0:W], OP.subtract)
            lnv = st32.tile([HPG, CH], F32, name="lnv", tag="lnv")
            nc.scalar.activation(lnv[:, 0:W], var[:, 0:W], AF.Ln,
                                 bias=eps_sb[:])
            rs = st16.tile([HPG, CH], F16, name="rs", tag="rs")
            nc.scalar.activation(rs[:, 0:W], lnv[:, 0:W], AF.Exp,
                                 scale=-0.5, bias=lnq_sb[:])
            murs = st16.tile([HPG, CH], F16, name="murs", tag="murs")
            nc.vector.scalar_tensor_tensor(
                murs[:, 0:W], mu[:, 0:W], -1.0, rs[:, 0:W],
                OP.mult, OP.mult)
            st["rs"], st["murs"] = rs, murs
        units.append(stats)
        for t in range(NT):
            def apply(t=t):
                rrep = ps_mi.tile([P, CH], F32, name="rrep", tag="misc")
                _mm(nc, rrep[:, 0:W], bcast_sb[:, t, :], st["rs"][:, 0:W],
                    True, True)
                mrep = ps_mi.tile([P, CH], F32, name="mrep", tag="misc")
                _mm(nc, mrep[:, 0:W], bcast_sb[:, t, :], st["murs"][:, 0:W],
                    True, True)
                nc.vector.tensor_tensor(
                    q_sb[t][:, coff:coff + W], q_sb[t][:, coff:coff + W],
                    rrep[:, 0:W], OP.mult)
                nc.vector.tensor_tensor(
                    q_sb[t][:, coff:coff + W], q_sb[t][:, coff:coff + W],
                    mrep[:, 0:W], OP.add)
            units.append(apply)
        return units

    # attention state
    o_t = {}     # qc -> [3 tiles [128, W] f16]
    den6 = {}    # qc -> [1, 6W] f32 staged denominator row
    po_cur = {}  # qc -> open po accumulator
    sp_state = {}

    # step order: qc0 solo, then qc1 interleaved 2:1 with qc2, then qc2 tail.
    qs0 = [(0, h, kg) for h in range(HPG) for kg in range(TT // KT_GRP)]
    qs1 = [(1, h, kg) for h in range(HPG) for kg in range(TT // KT_GRP)]
    qs2 = [(2, h, kg) for h in range(HPG) for kg in range(TT // KT_GRP)]
    merged = []
    i1 = i2 = 0
    while i1 < len(qs1) or i2 < len(qs2):
        for _ in range(2):
            if i1 < len(qs1):
                merged.append(qs1[i1])
                i1 += 1
        if i2 < len(qs2):
            merged.append(qs2[i2])
            i2 += 1
    steps = qs0 + merged
    inject = {0: lambda: qln_units(2),
              len(qs0): lambda: norm_outproj_units(0),
              len(qs0) + 72: lambda: norm_outproj_units(1)}

    def emit_sp(i):
        qc, h, kg = steps[i]
        coff, W = QCHUNKS[qc]
        t, off = h // 2, (h % 2) * HD
        sp = ps_sp.tile([P, KT_GRP * CH], F32, name="sp", tag="sp")
        for j in range(KT_GRP):
            kt = kg * KT_GRP + j
            _mm(nc, sp[:, j * W:(j + 1) * W],
                k_sb[t][off:off + HD, kt * P:(kt + 1) * P],
                q_sb[t][off:off + HD, coff:coff + W], True, True)
        e = e_pool.tile([P, KT_GRP * CH], F16, name="e", tag="e")
        sp_state[i] = (sp, e)

    def emit_exp(i):
        sp, e = sp_state[i]
        qc, h, kg = steps[i]
        W = QCHUNKS[qc][1]
        nc.scalar.activation(e[:, 0:KT_GRP * W], sp[:, 0:KT_GRP * W], AF.Exp)

    def emit_pv(i):
        qc, h, kg = steps[i]
        W = QCHUNKS[qc][1]
        sp, e = sp_state.pop(i)
        if kg == 0:
            po_cur[qc] = ps_po.tile([HD + 1, CH], F32, name="po", tag="po")
        po = po_cur[qc]
        for j in range(KT_GRP):
            kt = kg * KT_GRP + j
            _mm(nc, po[:, 0:W], v_sb[:, kt, h, :], e[:, j * W:(j + 1) * W],
                kt == 0, kt == TT - 1)
        if kg == TT // KT_GRP - 1:
            if h == 0:
                o_t[qc] = [o_pool.tile([P, CH], F16, name="ot", tag="ot")
                           for _ in range(NT)]
                den6[qc] = den_pool.tile([1, HPG * CH], F32, name="d6",
                                         tag="d6")
            t, off = h // 2, (h % 2) * HD
            nc.vector.tensor_copy(
                den6[qc][0:1, h * W:(h + 1) * W], po[HD:HD + 1, 0:W])
            nc.vector.tensor_copy(
                o_t[qc][t][off:off + HD, 0:W], po[0:HD, 0:W])

    def norm_outproj_units(qc):
        """Normalize by softmax denominators and project out; returns a
        list of small emission units to spread across attention steps."""
        coff, W = QCHUNKS[qc]
        units = []
        d6p = den_pool.tile([HPG, CH], F32, name="d6p", tag="d6p")
        d6r = den_pool.tile([HPG, CH], F16, name="d6r", tag="d6r")

        def recip():
            # scatter the staged [1, 6W] denominator row across partitions
            nc.sync.dma_start(d6p[0:HPG, 0:W], den6[qc][0:1, 0:HPG * W])
            with nc.allow_low_precision(reason="softmax denom recip in f16"):
                nc.vector.reciprocal(d6r[0:HPG, 0:W], d6p[0:HPG, 0:W])
        units.append(recip)
        for t in range(NT):
            def norm(t=t):
                rrep = ps_mi.tile([P, CH], F32, name="nrr", tag="misc")
                _mm(nc, rrep[:, 0:W], bcast_sb[:, t, :], d6r[:, 0:W],
                    True, True)
                nc.vector.tensor_tensor(
                    o_t[qc][t][:, 0:W], o_t[qc][t][:, 0:W], rrep[:, 0:W],
                    OP.mult)
            units.append(norm)
        for m in range(CT):
            def oproj(m=m):
                pp = ps_mi.tile([P, CH], F32, name="opp", tag="misc")
                for t in range(NT):
                    _mm(nc, pp[:, 0:W], wp_sb[:, t, m * P:(m + 1) * P],
                        o_t[qc][t][:, 0:W], t == 0, t == NT - 1)
                ot = out_pool.tile([P, CH], F32, name="oc", tag="oc")
                nc.vector.tensor_copy(ot[:, 0:W], pp[:, 0:W])
                nc.sync.dma_start(outT[m * P:(m + 1) * P, coff:coff + W],
                                  ot[:, 0:W])
            units.append(oproj)
        return units

    # ---------------- phase B emission with software pipelining --------
    qln(0)
    qln(1)
    fillers = []
    emit_sp(0)
    for i in range(len(steps)):
        if i in inject:
            fillers.extend(inject[i]())
        if i + 1 < len(steps):
            emit_sp(i + 1)
        emit_exp(i)
        emit_pv(i)
        if i % 2 == 1 and fillers:
            fillers.pop(0)()
    while fillers:
        fillers.pop(0)()
    for unit in norm_outproj_units(len(QCHUNKS) - 1):
        unit()

    for pool in (out_pool, den_pool, o_pool, e_pool, xq_pool,
                 ps_mi, ps_po, ps_sp, st16, st32, sq_pool, xkv_pool,
                 w_pool, bpool, cpool):
        pool.release()


def build_bass():
    nc = bass.Bass(trn_type="TRN2", debug=False, num_devices=NCORES)
    qxT = nc.dram_tensor("qxT", [C, MQ], F16, kind="ExternalInput").ap()
    kvxT = nc.dram_tensor("kvxT", [C, N], F16, kind="ExternalInput").ap()
    wq = nc.dram_tensor("wq", [C, CL], F16, kind="ExternalInput").ap()
    wk = nc.dram_tensor("wk", [C, CL], F16, kind="ExternalInput").ap()
    wv = nc.dram_tensor("wv", [C, CL], F16, kind="ExternalInput").ap()
    wp = nc.dram_tensor("wp", [CL, C], F16, kind="ExternalInput").ap()
    colsel = nc.dram_tensor("colsel", [P, NT, HPG], F16,
                            kind="ExternalInput").ap()
    bcast = nc.dram_tensor("bcast", [HPG, NT, P], F16,
                           kind="ExternalInput").ap()
    outT = nc.dram_tensor("outT", [C, MQ], F32, kind="ExternalOutput").ap()
    aps = (qxT, kvxT, wq, wk, wv, wp, colsel, bcast, outT)
    with _FixedTileContext(nc) as tc:
        _body(tc, aps)
    return nc


def make_in_maps(q_x, kv_x, attn_mask, Wq, Wkv, Wp):
    colsel = np.zeros((P, NT, HPG), np.float16)
    bcast = np.zeros((HPG, NT, P), np.float16)
    for t in range(NT):
        for pp in range(P):
            colsel[pp, t, 2 * t + pp // HD] = 1.0 / HD
            bcast[2 * t + pp // HD, t, pp] = 1.0

    mask = np.asarray(attn_mask, bool)
    in_maps = []
    for core in range(NCORES):
        b, g = core // G, core % G
        sl = slice(g * CL, (g + 1) * CL)
        idx = np.flatnonzero(mask[b])
        cnt = len(idx)
        assert cnt < MQ, f"mask count {cnt} exceeds padded width {MQ}"
        qxT_c = np.zeros((C, MQ), np.float16)
        qxT_c[:, :cnt] = q_x[b][idx].T
        in_maps.append({
            "qxT": qxT_c,
            "kvxT": np.ascontiguousarray(kv_x[b].T.astype(np.float16)),
            "wq": np.ascontiguousarray(Wq[sl].T.astype(np.float16)),
            "wk": np.ascontiguousarray(Wkv[sl].T.astype(np.float16)),
            "wv": np.ascontiguousarray(
                Wkv[C + g * CL:C + (g + 1) * CL].T.astype(np.float16)),
            "wp": np.ascontiguousarray(Wp[:, sl].T.astype(np.float16)),
            "colsel": colsel,
            "bcast": bcast,
        })
    return in_maps


_NC_CACHE = []


def get_nc():
    if not _NC_CACHE:
        _NC_CACHE.append(build_bass())
    return _NC_CACHE[0]


def kernel(q_x, kv_x, attn_mask, Wq, Wkv, qn_w, qn_b, kn_w, kn_b, Wp, bp,
           _profile=None):
    q_x = np.asarray(q_x, np.float32)
    kv_x = np.asarray(kv_x, np.float32)
    attn_mask = np.asarray(attn_mask, bool)
    Wq = np.asarray(Wq, np.float32)
    Wkv = np.asarray(Wkv, np.float32)
    Wp = np.asarray(Wp, np.float32)
    bp = np.asarray(bp, np.float32)
    if not (np.all(np.asarray(qn_w) == 1) and np.all(np.asarray(qn_b) == 0)
            and np.all(np.asarray(kn_w) == 1) and np.all(np.asarray(kn_b) == 0)):
        raise NotImplementedError("kernel specialized to identity q/k norms")

    nc = get_nc()
    in_maps = make_in_maps(q_x, kv_x, attn_mask, Wq, Wkv, Wp)
    res = bass_utils.run_bass_kernel_spmd(
        nc, in_maps, core_ids=list(range(NCORES)))
    if _profile is not None:
        _profile.append(res)
    out = np.empty((B, N, C), np.float32)
    for b in range(B):
        acc = res.results[G * b]["outT"] + res.results[G * b + 1]["outT"]
        idx = np.flatnonzero(attn_mask[b])
        cnt = len(idx)
        out[b, idx] = acc[:, :cnt].T + bp
        out[b, ~attn_mask[b]] = acc[:, cnt] + bp
    return out
